# revision 7
# baseline (speedup 1.0000x reference)
"""Trainium2 Bass kernel for nn_Complex_Only_46308337385506 (gnn_message_passing).

Math (derived + numerically validated against the jax reference):
  The per-edge orthonormal basis R (rows nU, nV, nJ) enters the output only
  through two per-edge scalars:
      gam = nJ_z = Jz/(|J|+eps)
      A1p = copysign(sqrt(Jx^2+Jy^2), gam+eps)/(|J|+eps)    (= -nU_z approx)
  With w = gam*Xz - A1p*Xx:
      Y0 = Wa@Xx + (Wa-Wc)@(A1p*w) + Wb@(gam*Xy)
      Y1 = Wa@Xy - Wb@(A1p*Xz + gam*Xx)
      Y2 = Wa@Xz + (Wc-Wa)@(gam*w) + Wb@(A1p*Xy)
  followed by the VN leaky-relu stage:
      d = Wd@Y (over channel dim), dot = <Y,d>_3, dn2 = <d,d>_3
      out = Y - 0.8*min(dot,0)/(dn2+eps) * d

Sharding: data-parallel over batch B=8 -> one batch per NeuronCore.
Per-core layout: supers of 1024 points; points are transposed on the PE
(pairs of feature blocks) so the E-contraction runs as [K<=128, N=512]
matmuls; stage-3 runs on [128, 512] tiles (two 512-pt groups stacked on
partitions).
"""

import math
import os
import numpy as np
from contextlib import ExitStack

import concourse.bass as bass
import concourse.bacc as bacc
import concourse.tile as tile
from concourse import mybir
from concourse import bass_utils

F32 = mybir.dt.float32
U32 = mybir.dt.uint32
AF = mybir.ActivationFunctionType
ALU = mybir.AluOpType

EPS = 1e-6
NEG = 0.2

B, C, E = 8, 16384, 64
SUPER = 1024           # points per super-iteration
NSUP = C // SUPER      # 16
GROUP = 512            # matmul free dim (points)
NCHUNK = 8             # 128-pt chunks per super


def _build_nc():
    nc = bacc.Bacc("TRN2", debug=False)

    XS = nc.dram_tensor("XS", [C, 192], F32, kind="ExternalInput").ap()
    JS = nc.dram_tensor("JS", [C, 192], F32, kind="ExternalInput").ap()
    WMM = nc.dram_tensor("WMM", [6, 128, 128], F32, kind="ExternalInput").ap()
    OUT = nc.dram_tensor("OUT", [64, 3, C], F32, kind="ExternalOutput").ap()

    with tile.TileContext(nc) as tc, ExitStack() as ctx:
        const = ctx.enter_context(tc.tile_pool(name="const", bufs=1))
        io = ctx.enter_context(tc.tile_pool(name="io", bufs=2))
        sa = ctx.enter_context(tc.tile_pool(name="sa", bufs=1))
        prodp = ctx.enter_context(tc.tile_pool(name="prodp", bufs=2))
        rhsp = ctx.enter_context(tc.tile_pool(name="rhsp", bufs=2))
        xsbp = ctx.enter_context(tc.tile_pool(name="xsbp", bufs=2))
        s3p = ctx.enter_context(tc.tile_pool(name="s3p", bufs=1))
        outp = ctx.enter_context(tc.tile_pool(name="outp", bufs=2))
        psT = ctx.enter_context(tc.tile_pool(name="psT", bufs=1, space="PSUM"))
        psY = ctx.enter_context(tc.tile_pool(name="psY", bufs=1, space="PSUM"))
        psD = ctx.enter_context(tc.tile_pool(name="psD", bufs=2, space="PSUM"))

        # bias constants for ACT
        eps_c = const.tile([128, 1], F32, tag="eps_c")
        ln8_c = const.tile([128, 1], F32, tag="ln8_c")
        nc.gpsimd.memset(eps_c[:], EPS)
        nc.gpsimd.memset(ln8_c[:], float(math.log(1.0 - NEG)))
        sgn_c = const.tile([128, 1], U32, tag="sgn_c")
        nc.gpsimd.memset(sgn_c[:], 0x80000000)

        # weights + identity, loaded once
        wsb = const.tile([128, 6, 128], F32)
        nc.sync.dma_start(wsb[:], WMM.rearrange("n p m -> p n m"))
        LW_A = wsb[:, 0, :]      # blkdiag(WaT, WaT)
        LW_2 = wsb[:, 1, :]      # blkdiag((Wa-Wc).T, (Wc-Wa).T)
        LW_B = wsb[:, 2, :]      # blkdiag(WbT, WbT)
        LW_1 = wsb[:, 3, 0:64]   # [WaT; -WbT], M=64
        LW_D = wsb[:, 4, :]      # blkdiag(WdT, WdT)
        IDT = wsb[:, 5, :]       # identity

        X3 = XS.rearrange("(u s p) w -> u p s w", p=128, s=NCHUNK)
        J3 = JS.rearrange("(u s p) w -> u p s w", p=128, s=NCHUNK)

        for u in range(NSUP):
            xs = io.tile([128, NCHUNK * 192], F32, tag="xs")
            js = io.tile([128, NCHUNK * 192], F32, tag="js")
            nc.sync.dma_start(
                xs[:].rearrange("p (s w) -> p s w", s=NCHUNK, w=192), X3[u])
            nc.sync.dma_start(
                js[:].rearrange("p (s w) -> p s w", s=NCHUNK, w=192), J3[u])
            xv = xs[:].rearrange("p (s e c) -> p s e c", s=NCHUNK, e=E, c=3)
            jv = js[:].rearrange("p (s e c) -> p s e c", s=NCHUNK, e=E, c=3)

            def v3(t):  # [128, 512] tile -> [128, 8, 64] view
                return t[:].rearrange("p (s e) -> p s e", s=NCHUNK, e=E)

            # ---- stage A: per-edge scalars gam, A1p --------------------
            jx2 = sa.tile([128, SUPER // 2], F32, tag="jx2")
            jy2 = sa.tile([128, SUPER // 2], F32, tag="jy2")
            jz2 = sa.tile([128, SUPER // 2], F32, tag="jz2")
            nc.scalar.activation(v3(jx2), jv[:, :, :, 0], AF.Square)
            nc.scalar.activation(v3(jy2), jv[:, :, :, 1], AF.Square)
            nc.scalar.activation(v3(jz2), jv[:, :, :, 2], AF.Square)
            q = sa.tile([128, SUPER // 2], F32, tag="q")
            n2 = sa.tile([128, SUPER // 2], F32, tag="n2")
            nc.vector.tensor_tensor(q[:], jx2[:], jy2[:], ALU.add)
            nc.vector.tensor_tensor(n2[:], q[:], jz2[:], ALU.add)
            r = sa.tile([128, SUPER // 2], F32, tag="r")
            rho = sa.tile([128, SUPER // 2], F32, tag="rho")
            nc.scalar.activation(r[:], n2[:], AF.Sqrt)
            nc.scalar.activation(rho[:], q[:], AF.Sqrt)
            t_ = sa.tile([128, SUPER // 2], F32, tag="t_")
            nc.vector.tensor_scalar(r[:], r[:], EPS, None, ALU.add)
            nc.vector.reciprocal_approx_fast(t_[:], r[:])
            gam = sa.tile([128, SUPER // 2], F32, tag="gam")
            nc.vector.tensor_tensor(v3(gam), jv[:, :, :, 2], v3(t_), ALU.mult)
            h = sa.tile([128, SUPER // 2], F32, tag="h")
            nc.vector.tensor_scalar(h[:], gam[:], EPS, None, ALU.add)
            rs = sa.tile([128, SUPER // 2], F32, tag="rs")
            nc.vector.scalar_tensor_tensor(
                rs[:].bitcast(U32), h[:].bitcast(U32), sgn_c[:],
                rho[:].bitcast(U32), ALU.bitwise_and, ALU.bitwise_or)
            a1 = sa.tile([128, SUPER // 2], F32, tag="a1")
            nc.vector.tensor_tensor(a1[:], rs[:], t_[:], ALU.mult)

            # ---- stage B: feature blocks into PROD slots ---------------
            # slots: 0=Xx 1=Xz 2=A1p*w 3=gam*w 4=gam*Xy 5=A1p*Xy 6=Xy 7=c8
            prod = prodp.tile([128, NCHUNK, 8, E], F32, tag="prod")
            nc.scalar.activation(prod[:, :, 0, :], xv[:, :, :, 0], AF.Copy)
            nc.scalar.activation(prod[:, :, 1, :], xv[:, :, :, 2], AF.Copy)
            nc.scalar.activation(prod[:, :, 6, :], xv[:, :, :, 1], AF.Copy)
            m1 = sa.tile([128, SUPER // 2], F32, tag="m1")
            m2 = sa.tile([128, SUPER // 2], F32, tag="m2")
            wt = sa.tile([128, SUPER // 2], F32, tag="wt")
            nc.vector.tensor_tensor(v3(m1), v3(gam), xv[:, :, :, 2], ALU.mult)
            nc.vector.tensor_tensor(v3(m2), v3(a1), xv[:, :, :, 0], ALU.mult)
            nc.vector.tensor_tensor(wt[:], m1[:], m2[:], ALU.subtract)
            nc.vector.tensor_tensor(prod[:, :, 2, :], v3(a1), v3(wt), ALU.mult)
            nc.vector.tensor_tensor(prod[:, :, 3, :], v3(gam), v3(wt), ALU.mult)
            nc.vector.tensor_tensor(prod[:, :, 4, :], v3(gam), xv[:, :, :, 1],
                                    ALU.mult)
            nc.vector.tensor_tensor(prod[:, :, 5, :], v3(a1), xv[:, :, :, 1],
                                    ALU.mult)
            m3 = sa.tile([128, SUPER // 2], F32, tag="m3")
            m4 = sa.tile([128, SUPER // 2], F32, tag="m4")
            nc.vector.tensor_tensor(v3(m3), v3(a1), xv[:, :, :, 2], ALU.mult)
            nc.vector.tensor_tensor(v3(m4), v3(gam), xv[:, :, :, 0], ALU.mult)
            nc.vector.tensor_tensor(prod[:, :, 7, :], v3(m3), v3(m4), ALU.add)

            # ---- per group: transpose, matmuls, Y copies ---------------
            xsb = xsbp.tile([128, 3, GROUP], F32, tag="xsb")
            for g in range(2):
                tpX = psT.tile([128, GROUP], F32, tag="tpX")
                tp1 = psT.tile([128, GROUP], F32, tag="tp1")
                tp2 = psT.tile([128, GROUP], F32, tag="tp2")
                tp3 = psT.tile([128, GROUP], F32, tag="tp3")
                for k in range(4):
                    s = 4 * g + k
                    sl = slice(128 * k, 128 * (k + 1))
                    nc.tensor.transpose(tpX[:, sl], prod[:, s, 0:2, :], IDT)
                    nc.tensor.transpose(tp1[:, sl], prod[:, s, 2:4, :], IDT)
                    nc.tensor.transpose(tp2[:, sl], prod[:, s, 4:6, :], IDT)
                    nc.tensor.transpose(tp3[:, sl], prod[:, s, 6:8, :], IDT)
                rhX = rhsp.tile([128, GROUP], F32, tag="rhX")
                rh1 = rhsp.tile([128, GROUP], F32, tag="rh1")
                rh2 = rhsp.tile([128, GROUP], F32, tag="rh2")
                rh3 = rhsp.tile([128, GROUP], F32, tag="rh3")
                nc.vector.tensor_copy(rhX[:], tpX[:])
                nc.vector.tensor_copy(rh1[:], tp1[:])
                nc.scalar.activation(rh2[:], tp2[:], AF.Copy)
                nc.scalar.activation(rh3[:], tp3[:], AF.Copy)

                pA = psY.tile([128, GROUP], F32, tag="pA")
                pB = psY.tile([64, GROUP], F32, tag="pB")
                nc.tensor.matmul(pA[:], LW_A, rhX[:], start=True, stop=False)
                nc.tensor.matmul(pA[:], LW_2, rh1[:], start=False, stop=False)
                nc.tensor.matmul(pA[:], LW_B, rh2[:], start=False, stop=True)
                nc.tensor.matmul(pB[:], LW_1, rh3[:], start=True, stop=True)

                ro = slice(64 * g, 64 * (g + 1))
                nc.vector.tensor_copy(xsb[ro, 0, :], pA[0:64, :])
                nc.vector.tensor_copy(xsb[ro, 2, :], pA[64:128, :])
                nc.vector.tensor_copy(xsb[ro, 1, :], pB[:])

            # ---- Wd stage + VN leaky relu ------------------------------
            dsb = s3p.tile([128, 3, GROUP], F32, tag="dsb")
            for i in range(3):
                pd = psD.tile([128, GROUP], F32, tag="pd")
                nc.tensor.matmul(pd[:], LW_D, xsb[:, i, :], start=True,
                                 stop=True)
                nc.scalar.activation(dsb[:, i, :], pd[:], AF.Copy)

            dot = s3p.tile([128, GROUP], F32, tag="dot")
            tmp = s3p.tile([128, GROUP], F32, tag="tmp")
            nc.vector.tensor_tensor(dot[:], xsb[:, 0, :], dsb[:, 0, :],
                                    ALU.mult)
            nc.vector.tensor_tensor(tmp[:], xsb[:, 1, :], dsb[:, 1, :],
                                    ALU.mult)
            nc.vector.tensor_tensor(dot[:], dot[:], tmp[:], ALU.add)
            nc.vector.tensor_tensor(tmp[:], xsb[:, 2, :], dsb[:, 2, :],
                                    ALU.mult)
            nc.vector.tensor_tensor(dot[:], dot[:], tmp[:], ALU.add)

            ds0 = s3p.tile([128, GROUP], F32, tag="ds0")
            ds1 = s3p.tile([128, GROUP], F32, tag="ds1")
            dn2 = s3p.tile([128, GROUP], F32, tag="dn2")
            nc.scalar.activation(ds0[:], dsb[:, 0, :], AF.Square)
            nc.scalar.activation(ds1[:], dsb[:, 1, :], AF.Square)
            nc.scalar.activation(dn2[:], dsb[:, 2, :], AF.Square)
            nc.vector.tensor_tensor(dn2[:], dn2[:], ds0[:], ALU.add)
            nc.vector.tensor_tensor(dn2[:], dn2[:], ds1[:], ALU.add)

            lnv = s3p.tile([128, GROUP], F32, tag="lnv")
            rec = s3p.tile([128, GROUP], F32, tag="rec")
            nc.scalar.activation(lnv[:], dn2[:], AF.Ln, bias=eps_c[:])
            nc.scalar.activation(rec[:], lnv[:], AF.Exp, scale=-1.0,
                                 bias=ln8_c[:])
            s2 = s3p.tile([128, GROUP], F32, tag="s2")
            nc.vector.tensor_scalar(s2[:], dot[:], 0.0, None, ALU.min)
            nc.vector.tensor_tensor(s2[:], s2[:], rec[:], ALU.mult)

            ot = outp.tile([128, 3, GROUP], F32, tag="ot")
            for i in range(3):
                mi = s3p.tile([128, GROUP], F32, tag="mi")
                nc.vector.tensor_tensor(mi[:], s2[:], dsb[:, i, :], ALU.mult)
                nc.vector.tensor_tensor(ot[:, i, :], xsb[:, i, :], mi[:],
                                        ALU.subtract)

            c0 = u * SUPER
            nc.sync.dma_start(OUT[:, :, c0:c0 + GROUP], ot[0:64])
            nc.sync.dma_start(OUT[:, :, c0 + GROUP:c0 + SUPER], ot[64:128])

    nc.compile()
    return nc


_NC = None


def _get_nc():
    global _NC
    if _NC is None:
        _NC = _build_nc()
    return _NC


def _weight_stack(Wa, Wb, Wc, Wd):
    Z = np.zeros((64, 64), np.float32)

    def blk(a, b):
        return np.block([[a, Z], [Z, b]]).astype(np.float32)

    WaT = Wa.T.astype(np.float32)
    WbT = Wb.T.astype(np.float32)
    W2nT = (Wa - Wc).T.astype(np.float32)
    W2T = (Wc - Wa).T.astype(np.float32)
    WdT = Wd.T.astype(np.float32)
    w = np.stack([
        blk(WaT, WaT),
        blk(W2nT, W2T),
        blk(WbT, WbT),
        np.block([[WaT, Z], [-WbT, Z]]).astype(np.float32),
        blk(WdT, WdT),
        np.eye(128, dtype=np.float32),
    ])
    return np.ascontiguousarray(w, np.float32)


def run_full(X, J, Wa, Wb, Wc, Wd, trace=False, trace_kwargs=None):
    nc = _get_nc()
    wmm = _weight_stack(Wa, Wb, Wc, Wd)
    in_maps = []
    for b in range(B):
        in_maps.append({
            "XS": np.ascontiguousarray(X[b].reshape(C, 192), np.float32),
            "JS": np.ascontiguousarray(J[b].reshape(C, 192), np.float32),
            "WMM": wmm,
        })
    res = bass_utils.run_bass_kernel_spmd(
        nc, in_maps, core_ids=list(range(B)), trace=trace,
        **(trace_kwargs or {}))
    out = np.stack([res.results[b]["OUT"] for b in range(B)])
    return out.astype(np.float32), res


def kernel(X, J, Wa, Wb, Wc, Wd):
    out, _ = run_full(X, J, Wa, Wb, Wc, Wd)
    return out


# revision 11
# speedup vs baseline: 1.2163x; 1.2163x over previous
"""Trainium2 Bass kernel for nn_Complex_Only_46308337385506 (gnn_message_passing).

Math (derived + numerically validated against the jax reference):
  The per-edge orthonormal basis R (rows nU, nV, nJ) enters the output only
  through two per-edge scalars:
      gam = nJ_z = Jz/(|J|+eps)
      A1p = copysign(sqrt(Jx^2+Jy^2), gam+eps)/(|J|+eps)    (= -nU_z approx)
  With w = gam*Xz - A1p*Xx:
      Y0 = Wa@Xx + (Wa-Wc)@(A1p*w) + Wb@(gam*Xy)
      Y1 = Wa@Xy - Wb@(A1p*Xz + gam*Xx)
      Y2 = Wa@Xz + (Wc-Wa)@(gam*w) + Wb@(A1p*Xy)
  followed by the VN leaky-relu stage:
      d = Wd@Y (over channel dim), dot = <Y,d>_3, dn2 = <d,d>_3
      out = Y - 0.8*min(dot,0)/(dn2+eps) * d

Sharding: data-parallel over batch B=8 -> one batch per NeuronCore.
Per-core layout: supers of 1024 points; points are transposed on the PE
(pairs of feature blocks) so the E-contraction runs as [K<=128, N=512]
matmuls; stage-3 runs on [128, 512] tiles (two 512-pt groups stacked on
partitions).
"""

import math
import os
import numpy as np
from contextlib import ExitStack

import concourse.bass as bass
import concourse.bacc as bacc
import concourse.tile as tile
from concourse import mybir
from concourse import bass_utils

F32 = mybir.dt.float32
U32 = mybir.dt.uint32
AF = mybir.ActivationFunctionType
ALU = mybir.AluOpType

EPS = 1e-6
NEG = 0.2

B, C, E = 8, 16384, 64
SUPER = 1024           # points per super-iteration
NSUP = C // SUPER      # 16
GROUP = 512            # matmul free dim (points)
NCHUNK = 8             # 128-pt chunks per super


def _pin_act_table_set(arch: str):
    """Steer the ACT table-set chooser: all funcs this kernel uses must
    first-match natural_log_exp_and_others, so exactly one table load is
    emitted (the chooser first-matches in act_info.json order)."""
    from concourse import hw_specs
    tables = hw_specs.get_activation_tables(arch)  # cached dict, mutate in place
    mine = {AF.Ln, AF.Exp, AF.Square, AF.Copy, AF.Identity}
    for name, funcs in tables.items():
        if name != "natural_log_exp_and_others":
            funcs -= mine


def _build_nc():
    nc = bacc.Bacc("TRN2", debug=False)
    _pin_act_table_set(nc.m.arch)

    XS = nc.dram_tensor("XS", [C, 192], F32, kind="ExternalInput").ap()
    JS = nc.dram_tensor("JS", [C, 192], F32, kind="ExternalInput").ap()
    WMM = nc.dram_tensor("WMM", [6, 128, 128], F32, kind="ExternalInput").ap()
    OUT = nc.dram_tensor("OUT", [64, 3, C], F32, kind="ExternalOutput").ap()

    with tile.TileContext(nc) as tc, ExitStack() as ctx:
        const = ctx.enter_context(tc.tile_pool(name="const", bufs=1))
        io = ctx.enter_context(tc.tile_pool(name="io", bufs=2))
        sa = ctx.enter_context(tc.tile_pool(name="sa", bufs=1))
        prodp = ctx.enter_context(tc.tile_pool(name="prodp", bufs=2))
        rhsp = ctx.enter_context(tc.tile_pool(name="rhsp", bufs=2))
        xsbp = ctx.enter_context(tc.tile_pool(name="xsbp", bufs=2))
        s3p = ctx.enter_context(tc.tile_pool(name="s3p", bufs=1))
        outp = ctx.enter_context(tc.tile_pool(name="outp", bufs=2))
        psT = ctx.enter_context(tc.tile_pool(name="psT", bufs=1, space="PSUM"))
        psY = ctx.enter_context(tc.tile_pool(name="psY", bufs=1, space="PSUM"))
        psD = ctx.enter_context(tc.tile_pool(name="psD", bufs=2, space="PSUM"))

        # bias constants for ACT
        eps_c = const.tile([128, 1], F32, tag="eps_c")
        ln8_c = const.tile([128, 1], F32, tag="ln8_c")
        nc.gpsimd.memset(eps_c[:], EPS)
        nc.gpsimd.memset(ln8_c[:], float(math.log(1.0 - NEG)))
        sgn_c = const.tile([128, 1], U32, tag="sgn_c")
        nc.gpsimd.memset(sgn_c[:], 0x80000000)

        # weights + identity, loaded once
        wsb = const.tile([128, 6, 128], F32)
        nc.sync.dma_start(wsb[:], WMM.rearrange("n p m -> p n m"))
        LW_A = wsb[:, 0, :]      # blkdiag(WaT, WaT)
        LW_2 = wsb[:, 1, :]      # blkdiag((Wa-Wc).T, (Wc-Wa).T)
        LW_B = wsb[:, 2, :]      # blkdiag(WbT, WbT)
        LW_1 = wsb[:, 3, 0:64]   # [WaT; -WbT], M=64
        LW_D = wsb[:, 4, :]      # blkdiag(WdT, WdT)
        IDT = wsb[:, 5, :]       # identity

        X3 = XS.rearrange("(u s p) w -> u p s w", p=128, s=NCHUNK)
        J3 = JS.rearrange("(u s p) w -> u p s w", p=128, s=NCHUNK)

        for u in range(NSUP):
            xs = io.tile([128, NCHUNK * 192], F32, tag="xs")
            js = io.tile([128, NCHUNK * 192], F32, tag="js")
            nc.sync.dma_start(
                xs[:].rearrange("p (s w) -> p s w", s=NCHUNK, w=192), X3[u])
            nc.sync.dma_start(
                js[:].rearrange("p (s w) -> p s w", s=NCHUNK, w=192), J3[u])
            xv = xs[:].rearrange("p (s e c) -> p s e c", s=NCHUNK, e=E, c=3)
            jv = js[:].rearrange("p (s e c) -> p s e c", s=NCHUNK, e=E, c=3)

            def v3(t):  # [128, 512] tile -> [128, 8, 64] view
                return t[:].rearrange("p (s e) -> p s e", s=NCHUNK, e=E)

            # ---- stage A: per-edge scalars gam, A1p --------------------
            jx2 = sa.tile([128, SUPER // 2], F32, tag="jx2")
            jy2 = sa.tile([128, SUPER // 2], F32, tag="jy2")
            jz2 = sa.tile([128, SUPER // 2], F32, tag="jz2")
            nc.scalar.activation(v3(jx2), jv[:, :, :, 0], AF.Square)
            nc.scalar.activation(v3(jy2), jv[:, :, :, 1], AF.Square)
            nc.scalar.activation(v3(jz2), jv[:, :, :, 2], AF.Square)
            q = sa.tile([128, SUPER // 2], F32, tag="q")
            n2 = sa.tile([128, SUPER // 2], F32, tag="n2")
            nc.vector.tensor_tensor(q[:], jx2[:], jy2[:], ALU.add)
            nc.vector.tensor_tensor(n2[:], q[:], jz2[:], ALU.add)
            # sqrt via exp(0.5*ln(x)): keeps every ACT func in the single
            # natural_log_exp_and_others table set (no table thrashing)
            r = sa.tile([128, SUPER // 2], F32, tag="r")
            rho = sa.tile([128, SUPER // 2], F32, tag="rho")
            nc.scalar.activation(r[:], n2[:], AF.Ln)
            nc.scalar.activation(r[:], r[:], AF.Exp, scale=0.5)
            nc.scalar.activation(rho[:], q[:], AF.Ln)
            nc.scalar.activation(rho[:], rho[:], AF.Exp, scale=0.5)
            t_ = sa.tile([128, SUPER // 2], F32, tag="t_")
            nc.vector.tensor_scalar(r[:], r[:], EPS, None, ALU.add)
            nc.vector.reciprocal_approx_fast(t_[:], r[:])
            gam = sa.tile([128, SUPER // 2], F32, tag="gam")
            nc.vector.tensor_tensor(v3(gam), jv[:, :, :, 2], v3(t_), ALU.mult)
            h = sa.tile([128, SUPER // 2], F32, tag="h")
            nc.vector.tensor_scalar(h[:], gam[:], EPS, None, ALU.add)
            rs = sa.tile([128, SUPER // 2], F32, tag="rs")
            nc.vector.scalar_tensor_tensor(
                rs[:].bitcast(U32), h[:].bitcast(U32), sgn_c[:],
                rho[:].bitcast(U32), ALU.bitwise_and, ALU.bitwise_or)
            a1 = sa.tile([128, SUPER // 2], F32, tag="a1")
            nc.vector.tensor_tensor(a1[:], rs[:], t_[:], ALU.mult)

            # ---- stage B: feature blocks into PROD slots ---------------
            # slots: 0=Xx 1=Xz 2=A1p*w 3=gam*w 4=gam*Xy 5=A1p*Xy 6=Xy 7=c8
            prod = prodp.tile([128, NCHUNK, 8, E], F32, tag="prod")
            nc.scalar.activation(prod[:, :, 0, :], xv[:, :, :, 0], AF.Copy)
            nc.scalar.activation(prod[:, :, 1, :], xv[:, :, :, 2], AF.Copy)
            nc.scalar.activation(prod[:, :, 6, :], xv[:, :, :, 1], AF.Copy)
            m1 = sa.tile([128, SUPER // 2], F32, tag="m1")
            m2 = sa.tile([128, SUPER // 2], F32, tag="m2")
            wt = sa.tile([128, SUPER // 2], F32, tag="wt")
            nc.vector.tensor_tensor(v3(m1), v3(gam), xv[:, :, :, 2], ALU.mult)
            nc.vector.tensor_tensor(v3(m2), v3(a1), xv[:, :, :, 0], ALU.mult)
            nc.vector.tensor_tensor(wt[:], m1[:], m2[:], ALU.subtract)
            nc.vector.tensor_tensor(prod[:, :, 2, :], v3(a1), v3(wt), ALU.mult)
            nc.vector.tensor_tensor(prod[:, :, 3, :], v3(gam), v3(wt), ALU.mult)
            nc.gpsimd.tensor_tensor(prod[:, :, 4, :], v3(gam), xv[:, :, :, 1],
                                    ALU.mult)
            nc.gpsimd.tensor_tensor(prod[:, :, 5, :], v3(a1), xv[:, :, :, 1],
                                    ALU.mult)
            m3 = sa.tile([128, SUPER // 2], F32, tag="m3")
            m4 = sa.tile([128, SUPER // 2], F32, tag="m4")
            nc.gpsimd.tensor_tensor(v3(m3), v3(a1), xv[:, :, :, 2], ALU.mult)
            nc.gpsimd.tensor_tensor(v3(m4), v3(gam), xv[:, :, :, 0], ALU.mult)
            nc.vector.tensor_tensor(prod[:, :, 7, :], v3(m3), v3(m4), ALU.add)

            # ---- per group: transpose, matmuls, Y copies ---------------
            xsb = xsbp.tile([128, 3, GROUP], F32, tag="xsb")
            for g in range(2):
                tpX = psT.tile([128, GROUP], F32, tag="tpX")
                tp1 = psT.tile([128, GROUP], F32, tag="tp1")
                tp2 = psT.tile([128, GROUP], F32, tag="tp2")
                tp3 = psT.tile([128, GROUP], F32, tag="tp3")
                for k in range(4):
                    s = 4 * g + k
                    sl = slice(128 * k, 128 * (k + 1))
                    nc.tensor.transpose(tpX[:, sl], prod[:, s, 0:2, :], IDT)
                    nc.tensor.transpose(tp1[:, sl], prod[:, s, 2:4, :], IDT)
                    nc.tensor.transpose(tp2[:, sl], prod[:, s, 4:6, :], IDT)
                    nc.tensor.transpose(tp3[:, sl], prod[:, s, 6:8, :], IDT)
                rhX = rhsp.tile([128, GROUP], F32, tag="rhX")
                rh1 = rhsp.tile([128, GROUP], F32, tag="rh1")
                rh2 = rhsp.tile([128, GROUP], F32, tag="rh2")
                rh3 = rhsp.tile([128, GROUP], F32, tag="rh3")
                nc.vector.tensor_copy(rhX[:], tpX[:])
                nc.vector.tensor_copy(rh1[:], tp1[:])
                nc.scalar.activation(rh2[:], tp2[:], AF.Copy)
                nc.scalar.activation(rh3[:], tp3[:], AF.Copy)

                pA = psY.tile([128, GROUP], F32, tag="pA")
                pB = psY.tile([64, GROUP], F32, tag="pB")
                nc.tensor.matmul(pA[:], LW_A, rhX[:], start=True, stop=False)
                nc.tensor.matmul(pA[:], LW_2, rh1[:], start=False, stop=False)
                nc.tensor.matmul(pA[:], LW_B, rh2[:], start=False, stop=True)
                nc.tensor.matmul(pB[:], LW_1, rh3[:], start=True, stop=True)

                ro = slice(64 * g, 64 * (g + 1))
                nc.vector.tensor_copy(xsb[ro, 0, :], pA[0:64, :])
                nc.vector.tensor_copy(xsb[ro, 2, :], pA[64:128, :])
                nc.vector.tensor_copy(xsb[ro, 1, :], pB[:])

            # ---- Wd stage + VN leaky relu ------------------------------
            dsb = s3p.tile([128, 3, GROUP], F32, tag="dsb")
            for i in range(3):
                pd = psD.tile([128, GROUP], F32, tag="pd")
                nc.tensor.matmul(pd[:], LW_D, xsb[:, i, :], start=True,
                                 stop=True)
                nc.scalar.activation(dsb[:, i, :], pd[:], AF.Copy)

            dot = s3p.tile([128, GROUP], F32, tag="dot")
            tmp = s3p.tile([128, GROUP], F32, tag="tmp")
            nc.vector.tensor_tensor(dot[:], xsb[:, 0, :], dsb[:, 0, :],
                                    ALU.mult)
            nc.vector.tensor_tensor(tmp[:], xsb[:, 1, :], dsb[:, 1, :],
                                    ALU.mult)
            nc.vector.tensor_tensor(dot[:], dot[:], tmp[:], ALU.add)
            nc.vector.tensor_tensor(tmp[:], xsb[:, 2, :], dsb[:, 2, :],
                                    ALU.mult)
            nc.vector.tensor_tensor(dot[:], dot[:], tmp[:], ALU.add)

            ds0 = s3p.tile([128, GROUP], F32, tag="ds0")
            ds1 = s3p.tile([128, GROUP], F32, tag="ds1")
            dn2 = s3p.tile([128, GROUP], F32, tag="dn2")
            nc.scalar.activation(ds0[:], dsb[:, 0, :], AF.Square)
            nc.scalar.activation(ds1[:], dsb[:, 1, :], AF.Square)
            nc.scalar.activation(dn2[:], dsb[:, 2, :], AF.Square)
            nc.vector.tensor_tensor(dn2[:], dn2[:], ds0[:], ALU.add)
            nc.vector.tensor_tensor(dn2[:], dn2[:], ds1[:], ALU.add)

            lnv = s3p.tile([128, GROUP], F32, tag="lnv")
            rec = s3p.tile([128, GROUP], F32, tag="rec")
            nc.scalar.activation(lnv[:], dn2[:], AF.Ln, bias=eps_c[:])
            nc.scalar.activation(rec[:], lnv[:], AF.Exp, scale=-1.0,
                                 bias=ln8_c[:])
            s2 = s3p.tile([128, GROUP], F32, tag="s2")
            nc.vector.scalar_tensor_tensor(s2[:], dot[:], 0.0, rec[:],
                                           ALU.min, ALU.mult)

            ot = outp.tile([128, 3, GROUP], F32, tag="ot")
            for i in range(3):
                mi = s3p.tile([128, GROUP], F32, tag=f"mi{i}")
                nc.gpsimd.tensor_tensor(mi[:], s2[:], dsb[:, i, :], ALU.mult)
                nc.vector.tensor_tensor(ot[:, i, :], xsb[:, i, :], mi[:],
                                        ALU.subtract)

            c0 = u * SUPER
            nc.sync.dma_start(OUT[:, :, c0:c0 + GROUP], ot[0:64])
            nc.sync.dma_start(OUT[:, :, c0 + GROUP:c0 + SUPER], ot[64:128])

    nc.compile()
    return nc


_NC = None


def _get_nc():
    global _NC
    if _NC is None:
        _NC = _build_nc()
    return _NC


def _weight_stack(Wa, Wb, Wc, Wd):
    Z = np.zeros((64, 64), np.float32)

    def blk(a, b):
        return np.block([[a, Z], [Z, b]]).astype(np.float32)

    WaT = Wa.T.astype(np.float32)
    WbT = Wb.T.astype(np.float32)
    W2nT = (Wa - Wc).T.astype(np.float32)
    W2T = (Wc - Wa).T.astype(np.float32)
    WdT = Wd.T.astype(np.float32)
    w = np.stack([
        blk(WaT, WaT),
        blk(W2nT, W2T),
        blk(WbT, WbT),
        np.block([[WaT, Z], [-WbT, Z]]).astype(np.float32),
        blk(WdT, WdT),
        np.eye(128, dtype=np.float32),
    ])
    return np.ascontiguousarray(w, np.float32)


def run_full(X, J, Wa, Wb, Wc, Wd, trace=False, trace_kwargs=None):
    nc = _get_nc()
    wmm = _weight_stack(Wa, Wb, Wc, Wd)
    in_maps = []
    for b in range(B):
        in_maps.append({
            "XS": np.ascontiguousarray(X[b].reshape(C, 192), np.float32),
            "JS": np.ascontiguousarray(J[b].reshape(C, 192), np.float32),
            "WMM": wmm,
        })
    res = bass_utils.run_bass_kernel_spmd(
        nc, in_maps, core_ids=list(range(B)), trace=trace,
        **(trace_kwargs or {}))
    out = np.stack([res.results[b]["OUT"] for b in range(B)])
    return out.astype(np.float32), res


def kernel(X, J, Wa, Wb, Wc, Wd):
    out, _ = run_full(X, J, Wa, Wb, Wc, Wd)
    return out


# revision 21
# speedup vs baseline: 1.7129x; 1.4083x over previous
"""Trainium2 Bass kernel for nn_Complex_Only_46308337385506 (gnn_message_passing).

Math (derived + numerically validated against the jax reference):
  The per-edge orthonormal basis R (rows nU, nV, nJ) enters the output only
  through two per-edge scalars:
      gam = nJ_z = Jz/(|J|+eps)
      A1p = copysign(sqrt(Jx^2+Jy^2), gam+eps)/(|J|+eps)    (= -nU_z approx)
  With w = gam*Xz - A1p*Xx:
      Y0 = Wa@Xx + (Wa-Wc)@(A1p*w) + Wb@(gam*Xy)
      Y1 = Wa@Xy - Wb@(A1p*Xz + gam*Xx)
      Y2 = Wa@Xz + (Wc-Wa)@(gam*w) + Wb@(A1p*Xy)
  followed by the VN leaky-relu stage:
      d = Wd@Y (over channel dim), dot = <Y,d>_3, dn2 = <d,d>_3
      out = Y - 0.8*min(dot,0)/(dn2+eps) * d

Sharding: data-parallel over batch B=8 -> one batch per NeuronCore.
Per-core layout: supers of 1024 points; points are transposed on the PE
(pairs of feature blocks) so the E-contraction runs as [K<=128, N=512]
matmuls; stage-3 runs on [128, 512] tiles (two 512-pt groups stacked on
partitions).
"""

import math
import os
import numpy as np
from contextlib import ExitStack

import concourse.bass as bass
import concourse.bacc as bacc
import concourse.tile as tile
from concourse import mybir
from concourse import bass_utils

F32 = mybir.dt.float32
F32R = mybir.dt.float32r
U32 = mybir.dt.uint32
AF = mybir.ActivationFunctionType
ALU = mybir.AluOpType

EPS = 1e-6
NEG = 0.2

B, C, E = 8, 16384, 64
SUPER = 1024           # points per super-iteration
NSUP = C // SUPER      # 16
GROUP = 512            # matmul free dim (points)
NCHUNK = 8             # 128-pt chunks per super


_CUSTOM_OPS = {}


def _register_custom_dve_ops():
    """Register two fused DVE ops (module-level, idempotent):
      SQSUM_ANT: out = Src0^2 + Src1^2
      ADDSQ_ANT: out = Src0 + Src1^2
    Replaces {2x ACT Square + 1 DVE add} chains with one DVE pass each."""
    if _CUSTOM_OPS:
        return _CUSTOM_OPS
    import numpy as _np
    from concourse import dve_ops
    from concourse.dve_spec import Spec, Src0, Src1, lower, sq, _has_src1
    from concourse.dve_uop import DveOpSpec
    from concourse.dve_table_gen import dve_ver_for

    def make(name, body, ref):
        spec = Spec(body=body, reference=ref)
        opcode = dve_ops._CUSTOM_DVE_ROW_BASE + len(dve_ops.OPS)
        shas = {}
        for ver in ("v3", "v4"):
            try:
                s = DveOpSpec(name=name, opcode=opcode,
                              uops=lower(spec, ver=ver),
                              rd1_en=_has_src1(spec))
                shas[ver] = s.sha(ver)
            except Exception:
                pass
        op = dve_ops.DveOp(name, spec, subdim=False, uops_sha=shas)
        dve_ops.OPS.append(op)
        dve_ops.CUSTOM_DVE_SPECS[name] = spec
        dve_ops._SUB_OPCODE_FOR_NAME[name] = opcode
        assert opcode < 0x20
        return op

    _CUSTOM_OPS["SQSUM"] = make(
        "SQSUM_ANT", sq(Src0) + sq(Src1),
        lambda in0, in1, s0, s1, imm2:
            (in0.astype(_np.float32) * in0 + in1.astype(_np.float32) * in1))
    _CUSTOM_OPS["ADDSQ"] = make(
        "ADDSQ_ANT", Src0 + sq(Src1),
        lambda in0, in1, s0, s1, imm2:
            in0.astype(_np.float32) + in1.astype(_np.float32) * in1)
    return _CUSTOM_OPS


def _pin_act_table_set(arch: str):
    """Steer the ACT table-set chooser: all funcs this kernel uses must
    first-match natural_log_exp_and_others, so exactly one table load is
    emitted (the chooser first-matches in act_info.json order)."""
    from concourse import hw_specs
    tables = hw_specs.get_activation_tables(arch)  # cached dict, mutate in place
    mine = {AF.Ln, AF.Exp, AF.Square, AF.Copy, AF.Identity}
    for name, funcs in tables.items():
        if name != "natural_log_exp_and_others":
            funcs -= mine


def _build_nc():
    global OPS
    OPS = _register_custom_dve_ops()
    nc = bacc.Bacc("TRN2", debug=False)
    _pin_act_table_set(nc.m.arch)

    XS = nc.dram_tensor("XS", [C, 192], F32, kind="ExternalInput").ap()
    JS = nc.dram_tensor("JS", [C, 192], F32, kind="ExternalInput").ap()
    WMM = nc.dram_tensor("WMM", [6, 128, 128], F32, kind="ExternalInput").ap()
    OUT = nc.dram_tensor("OUT", [64, 3, C], F32, kind="ExternalOutput").ap()

    with tile.TileContext(nc) as tc, ExitStack() as ctx:
        const = ctx.enter_context(tc.tile_pool(name="const", bufs=1))
        io = ctx.enter_context(tc.tile_pool(name="io", bufs=2))
        sa = ctx.enter_context(tc.tile_pool(name="sa", bufs=1))
        prodp = ctx.enter_context(tc.tile_pool(name="prodp", bufs=2))
        rhsp = ctx.enter_context(tc.tile_pool(name="rhsp", bufs=2))
        xsbp = ctx.enter_context(tc.tile_pool(name="xsbp", bufs=2))
        s3p = ctx.enter_context(tc.tile_pool(name="s3p", bufs=1))
        outp = ctx.enter_context(tc.tile_pool(name="outp", bufs=2))
        psT = ctx.enter_context(tc.tile_pool(name="psT", bufs=1, space="PSUM"))
        psY = ctx.enter_context(tc.tile_pool(name="psY", bufs=1, space="PSUM"))
        psD = ctx.enter_context(tc.tile_pool(name="psD", bufs=2, space="PSUM"))

        # bias constants for ACT
        eps_c = const.tile([128, 1], F32, tag="eps_c")
        ln8_c = const.tile([128, 1], F32, tag="ln8_c")
        nc.gpsimd.memset(eps_c[:], EPS)
        nc.gpsimd.memset(ln8_c[:], float(math.log(1.0 - NEG)))
        sgn_c = const.tile([128, 1], U32, tag="sgn_c")
        nc.gpsimd.memset(sgn_c[:], 0x80000000)

        # weights + identity, loaded once
        wsb = const.tile([128, 6, 128], F32)
        nc.sync.dma_start(wsb[:], WMM.rearrange("n p m -> p n m"))
        LW_D = wsb[:, 4, :]      # blkdiag(WdT, WdT)
        IDT = wsb[:, 5, :]       # identity
        # f32r-rounded copies of the Y-matmul weights (PE full rate at N>=256)
        wsbr = const.tile([128, 4, 128], F32R, tag="wsbr")
        for k in range(4):
            nc.scalar.activation(wsbr[:, k, :], wsb[:, k, :], AF.Copy)
        LW_A = wsbr[:, 0, :]     # blkdiag(WaT, WaT)
        LW_2 = wsbr[:, 1, :]     # blkdiag((Wa-Wc).T, (Wc-Wa).T)
        LW_B = wsbr[:, 2, :]     # blkdiag(WbT, WbT)
        LW_1 = wsbr[:, 3, 0:64]  # [WaT; -WbT], M=64

        X3 = XS.rearrange("(u s p) w -> u p s w", p=128, s=NCHUNK)
        J3 = JS.rearrange("(u s p) w -> u p s w", p=128, s=NCHUNK)

        for u in range(NSUP):
            xs = io.tile([128, NCHUNK * 192], F32, tag="xs")
            js = io.tile([128, NCHUNK * 192], F32, tag="js")
            nc.sync.dma_start(
                xs[:].rearrange("p (s w) -> p s w", s=NCHUNK, w=192), X3[u])
            nc.sync.dma_start(
                js[:].rearrange("p (s w) -> p s w", s=NCHUNK, w=192), J3[u])
            xv = xs[:].rearrange("p (s e c) -> p s e c", s=NCHUNK, e=E, c=3)
            jv = js[:].rearrange("p (s e c) -> p s e c", s=NCHUNK, e=E, c=3)

            def v3(t):  # [128, 512] tile -> [128, 8, 64] view
                return t[:].rearrange("p (s e) -> p s e", s=NCHUNK, e=E)

            # ---- stage A: per-edge scalars gam, A1p --------------------
            q = sa.tile([128, SUPER // 2], F32, tag="q")
            n2 = sa.tile([128, SUPER // 2], F32, tag="n2")
            nc.vector._custom_dve(OPS["SQSUM"], out=v3(q),
                                  in0=jv[:, :, :, 0], in1=jv[:, :, :, 1])
            nc.vector._custom_dve(OPS["ADDSQ"], out=v3(n2),
                                  in0=v3(q), in1=jv[:, :, :, 2])
            # sqrt via exp(0.5*ln(x)): keeps every ACT func in the single
            # natural_log_exp_and_others table set (no table thrashing)
            r = sa.tile([128, SUPER // 2], F32, tag="r")
            rho = sa.tile([128, SUPER // 2], F32, tag="rho")
            nc.scalar.activation(r[:], n2[:], AF.Ln)
            nc.scalar.activation(r[:], r[:], AF.Exp, scale=0.5)
            nc.scalar.activation(rho[:], q[:], AF.Ln)
            nc.scalar.activation(rho[:], rho[:], AF.Exp, scale=0.5)
            t_ = sa.tile([128, SUPER // 2], F32, tag="t_")
            nc.vector.tensor_scalar(r[:], r[:], EPS, None, ALU.add)
            nc.vector.reciprocal_approx_fast(t_[:], r[:])
            gam = sa.tile([128, SUPER // 2], F32, tag="gam")
            nc.vector.tensor_tensor(v3(gam), jv[:, :, :, 2], v3(t_), ALU.mult)
            h = sa.tile([128, SUPER // 2], F32, tag="h")
            nc.vector.tensor_scalar(h[:], gam[:], EPS, None, ALU.add)
            rs = sa.tile([128, SUPER // 2], F32, tag="rs")
            nc.vector.scalar_tensor_tensor(
                rs[:].bitcast(U32), h[:].bitcast(U32), sgn_c[:],
                rho[:].bitcast(U32), ALU.bitwise_and, ALU.bitwise_or)
            a1 = sa.tile([128, SUPER // 2], F32, tag="a1")
            nc.vector.tensor_tensor(a1[:], rs[:], t_[:], ALU.mult)

            # ---- stage B: feature blocks into PROD slots ---------------
            # slots: 0=Xx 1=Xz 2=A1p*w 3=gam*w 4=gam*Xy 5=A1p*Xy 6=Xy 7=c8
            prod = prodp.tile([128, NCHUNK, 8, E], F32, tag="prod")
            nc.scalar.activation(prod[:, :, 0, :], xv[:, :, :, 0], AF.Copy)
            nc.scalar.activation(prod[:, :, 1, :], xv[:, :, :, 2], AF.Copy)
            nc.scalar.activation(prod[:, :, 6, :], xv[:, :, :, 1], AF.Copy)
            m1 = sa.tile([128, SUPER // 2], F32, tag="m1")
            m2 = sa.tile([128, SUPER // 2], F32, tag="m2")
            wt = sa.tile([128, SUPER // 2], F32, tag="wt")
            nc.vector.tensor_tensor(v3(m1), v3(gam), xv[:, :, :, 2], ALU.mult)
            nc.vector.tensor_tensor(v3(m2), v3(a1), xv[:, :, :, 0], ALU.mult)
            nc.vector.tensor_tensor(wt[:], m1[:], m2[:], ALU.subtract)
            nc.vector.tensor_tensor(prod[:, :, 2, :], v3(a1), v3(wt), ALU.mult)
            nc.vector.tensor_tensor(prod[:, :, 3, :], v3(gam), v3(wt), ALU.mult)
            nc.gpsimd.tensor_tensor(prod[:, :, 4, :], v3(gam), xv[:, :, :, 1],
                                    ALU.mult)
            nc.gpsimd.tensor_tensor(prod[:, :, 5, :], v3(a1), xv[:, :, :, 1],
                                    ALU.mult)
            m3 = sa.tile([128, SUPER // 2], F32, tag="m3")
            m4 = sa.tile([128, SUPER // 2], F32, tag="m4")
            nc.gpsimd.tensor_tensor(v3(m3), v3(a1), xv[:, :, :, 2], ALU.mult)
            nc.gpsimd.tensor_tensor(v3(m4), v3(gam), xv[:, :, :, 0], ALU.mult)
            nc.vector.tensor_tensor(prod[:, :, 7, :], v3(m3), v3(m4), ALU.add)

            # ---- per group: transpose, matmuls, Y copies ---------------
            xsb = xsbp.tile([128, 3, GROUP], F32, tag="xsb")
            for g in range(2):
                tpX = psT.tile([128, GROUP], F32, tag="tpX")
                tp1 = psT.tile([128, GROUP], F32, tag="tp1")
                tp2 = psT.tile([128, GROUP], F32, tag="tp2")
                tp3 = psT.tile([128, GROUP], F32, tag="tp3")
                for k in range(4):
                    s = 4 * g + k
                    sl = slice(128 * k, 128 * (k + 1))
                    nc.tensor.transpose(tpX[:, sl], prod[:, s, 0:2, :], IDT)
                    nc.tensor.transpose(tp1[:, sl], prod[:, s, 2:4, :], IDT)
                    nc.tensor.transpose(tp2[:, sl], prod[:, s, 4:6, :], IDT)
                    nc.tensor.transpose(tp3[:, sl], prod[:, s, 6:8, :], IDT)
                rhX = rhsp.tile([128, GROUP], F32R, tag="rhX")
                rh1 = rhsp.tile([128, GROUP], F32R, tag="rh1")
                rh2 = rhsp.tile([128, GROUP], F32R, tag="rh2")
                rh3 = rhsp.tile([128, GROUP], F32R, tag="rh3")
                nc.scalar.activation(rhX[:], tpX[:], AF.Copy)
                nc.scalar.activation(rh1[:], tp1[:], AF.Copy)
                nc.scalar.activation(rh2[:], tp2[:], AF.Copy)
                nc.scalar.activation(rh3[:], tp3[:], AF.Copy)

                pA = psY.tile([128, GROUP], F32, tag="pA")
                pB = psY.tile([64, GROUP], F32, tag="pB")
                nc.tensor.matmul(pA[:], LW_A, rhX[:], start=True, stop=False)
                nc.tensor.matmul(pA[:], LW_2, rh1[:], start=False, stop=False)
                nc.tensor.matmul(pA[:], LW_B, rh2[:], start=False, stop=True)
                nc.tensor.matmul(pB[:], LW_1, rh3[:], start=True, stop=True)

                ro = slice(64 * g, 64 * (g + 1))
                nc.scalar.activation(xsb[ro, 0, :], pA[0:64, :], AF.Copy)
                nc.vector.tensor_copy(xsb[ro, 2, :], pA[64:128, :])
                nc.scalar.activation(xsb[ro, 1, :], pB[:], AF.Copy)

            # ---- Wd stage + VN leaky relu ------------------------------
            dsb = s3p.tile([128, 3, GROUP], F32, tag="dsb")
            for i in range(3):
                pd = psD.tile([128, GROUP], F32, tag="pd")
                nc.tensor.matmul(pd[:], LW_D, xsb[:, i, :], start=True,
                                 stop=True)
                nc.scalar.activation(dsb[:, i, :], pd[:], AF.Copy)

            xd0 = s3p.tile([128, GROUP], F32, tag="xd0")
            xd1 = s3p.tile([128, GROUP], F32, tag="xd1")
            xd2 = s3p.tile([128, GROUP], F32, tag="xd2")
            dot = s3p.tile([128, GROUP], F32, tag="dot")
            nc.gpsimd.tensor_tensor(xd0[:], xsb[:, 0, :], dsb[:, 0, :],
                                    ALU.mult)
            nc.gpsimd.tensor_tensor(xd1[:], xsb[:, 1, :], dsb[:, 1, :],
                                    ALU.mult)
            nc.gpsimd.tensor_tensor(xd2[:], xsb[:, 2, :], dsb[:, 2, :],
                                    ALU.mult)
            nc.vector.tensor_tensor(dot[:], xd0[:], xd1[:], ALU.add)
            nc.vector.tensor_tensor(dot[:], dot[:], xd2[:], ALU.add)

            dn2 = s3p.tile([128, GROUP], F32, tag="dn2")
            nc.vector._custom_dve(OPS["SQSUM"], out=dn2[:],
                                  in0=dsb[:, 0, :], in1=dsb[:, 1, :])
            nc.vector._custom_dve(OPS["ADDSQ"], out=dn2[:],
                                  in0=dn2[:], in1=dsb[:, 2, :])

            lnv = s3p.tile([128, GROUP], F32, tag="lnv")
            rec = s3p.tile([128, GROUP], F32, tag="rec")
            nc.scalar.activation(lnv[:], dn2[:], AF.Ln, bias=eps_c[:])
            nc.scalar.activation(rec[:], lnv[:], AF.Exp, scale=-1.0,
                                 bias=ln8_c[:])
            s2 = s3p.tile([128, GROUP], F32, tag="s2")
            nc.vector.scalar_tensor_tensor(s2[:], dot[:], 0.0, rec[:],
                                           ALU.min, ALU.mult)

            ot = outp.tile([128, 3, GROUP], F32, tag="ot")
            for i in range(3):
                mi = s3p.tile([128, GROUP], F32, tag=f"mi{i}")
                nc.gpsimd.tensor_tensor(mi[:], s2[:], dsb[:, i, :], ALU.mult)
                nc.vector.tensor_tensor(ot[:, i, :], xsb[:, i, :], mi[:],
                                        ALU.subtract)

            c0 = u * SUPER
            nc.sync.dma_start(OUT[:, :, c0:c0 + GROUP], ot[0:64])
            nc.sync.dma_start(OUT[:, :, c0 + GROUP:c0 + SUPER], ot[64:128])

    nc.compile()
    return nc


_NC = None


def _get_nc():
    global _NC
    if _NC is None:
        _NC = _build_nc()
    return _NC


def _weight_stack(Wa, Wb, Wc, Wd):
    Z = np.zeros((64, 64), np.float32)

    def blk(a, b):
        return np.block([[a, Z], [Z, b]]).astype(np.float32)

    WaT = Wa.T.astype(np.float32)
    WbT = Wb.T.astype(np.float32)
    W2nT = (Wa - Wc).T.astype(np.float32)
    W2T = (Wc - Wa).T.astype(np.float32)
    WdT = Wd.T.astype(np.float32)
    w = np.stack([
        blk(WaT, WaT),
        blk(W2nT, W2T),
        blk(WbT, WbT),
        np.block([[WaT, Z], [-WbT, Z]]).astype(np.float32),
        blk(WdT, WdT),
        np.eye(128, dtype=np.float32),
    ])
    return np.ascontiguousarray(w, np.float32)


def run_full(X, J, Wa, Wb, Wc, Wd, trace=False, trace_kwargs=None):
    nc = _get_nc()
    wmm = _weight_stack(Wa, Wb, Wc, Wd)
    in_maps = []
    for b in range(B):
        in_maps.append({
            "XS": np.ascontiguousarray(X[b].reshape(C, 192), np.float32),
            "JS": np.ascontiguousarray(J[b].reshape(C, 192), np.float32),
            "WMM": wmm,
        })
    res = bass_utils.run_bass_kernel_spmd(
        nc, in_maps, core_ids=list(range(B)), trace=trace,
        **(trace_kwargs or {}))
    out = np.stack([res.results[b]["OUT"] for b in range(B)])
    return out.astype(np.float32), res


def kernel(X, J, Wa, Wb, Wc, Wd):
    out, _ = run_full(X, J, Wa, Wb, Wc, Wd)
    return out


# revision 24
# speedup vs baseline: 1.8957x; 1.1067x over previous
"""Trainium2 Bass kernel for nn_Complex_Only_46308337385506 (gnn_message_passing).

Math (derived + numerically validated against the jax reference):
  The per-edge orthonormal basis R (rows nU, nV, nJ) enters the output only
  through two per-edge scalars:
      gam = nJ_z = Jz/(|J|+eps)
      A1p = copysign(sqrt(Jx^2+Jy^2), gam+eps)/(|J|+eps)    (= -nU_z approx)
  With w = gam*Xz - A1p*Xx:
      Y0 = Wa@Xx + (Wa-Wc)@(A1p*w) + Wb@(gam*Xy)
      Y1 = Wa@Xy - Wb@(A1p*Xz + gam*Xx)
      Y2 = Wa@Xz + (Wc-Wa)@(gam*w) + Wb@(A1p*Xy)
  followed by the VN leaky-relu stage:
      d = Wd@Y (over channel dim), dot = <Y,d>_3, dn2 = <d,d>_3
      out = Y - 0.8*min(dot,0)/(dn2+eps) * d

Sharding: data-parallel over batch B=8 -> one batch per NeuronCore.
Per-core layout: supers of 1024 points; points are transposed on the PE
(pairs of feature blocks) so the E-contraction runs as [K<=128, N=512]
matmuls; stage-3 runs on [128, 512] tiles (two 512-pt groups stacked on
partitions).
"""

import math
import os
import numpy as np
from contextlib import ExitStack

import concourse.bass as bass
import concourse.bacc as bacc
import concourse.tile as tile
from concourse import mybir
from concourse import bass_utils

F32 = mybir.dt.float32
F32R = mybir.dt.float32r
U32 = mybir.dt.uint32
AF = mybir.ActivationFunctionType
ALU = mybir.AluOpType

EPS = 1e-6
NEG = 0.2

B, C, E = 8, 16384, 64
SUPER = 1024           # points per super-iteration
NSUP = C // SUPER      # 16
GROUP = 512            # matmul free dim (points)
NCHUNK = 8             # 128-pt chunks per super


_CUSTOM_OPS = {}


def _register_custom_dve_ops():
    """Register two fused DVE ops (module-level, idempotent):
      SQSUM_ANT: out = Src0^2 + Src1^2
      ADDSQ_ANT: out = Src0 + Src1^2
    Replaces {2x ACT Square + 1 DVE add} chains with one DVE pass each."""
    if _CUSTOM_OPS:
        return _CUSTOM_OPS
    import numpy as _np
    from concourse import dve_ops
    from concourse.dve_spec import Spec, Src0, Src1, lower, sq, _has_src1
    from concourse.dve_uop import DveOpSpec
    from concourse.dve_table_gen import dve_ver_for

    def make(name, body, ref):
        spec = Spec(body=body, reference=ref)
        opcode = dve_ops._CUSTOM_DVE_ROW_BASE + len(dve_ops.OPS)
        shas = {}
        for ver in ("v3", "v4"):
            try:
                s = DveOpSpec(name=name, opcode=opcode,
                              uops=lower(spec, ver=ver),
                              rd1_en=_has_src1(spec))
                shas[ver] = s.sha(ver)
            except Exception:
                pass
        op = dve_ops.DveOp(name, spec, subdim=False, uops_sha=shas)
        dve_ops.OPS.append(op)
        dve_ops.CUSTOM_DVE_SPECS[name] = spec
        dve_ops._SUB_OPCODE_FOR_NAME[name] = opcode
        assert opcode < 0x20
        return op

    _CUSTOM_OPS["SQSUM"] = make(
        "SQSUM_ANT", sq(Src0) + sq(Src1),
        lambda in0, in1, s0, s1, imm2:
            (in0.astype(_np.float32) * in0 + in1.astype(_np.float32) * in1))
    _CUSTOM_OPS["ADDSQ"] = make(
        "ADDSQ_ANT", Src0 + sq(Src1),
        lambda in0, in1, s0, s1, imm2:
            in0.astype(_np.float32) + in1.astype(_np.float32) * in1)
    return _CUSTOM_OPS


def _pin_act_table_set(arch: str):
    """Steer the ACT table-set chooser: all funcs this kernel uses must
    first-match natural_log_exp_and_others, so exactly one table load is
    emitted (the chooser first-matches in act_info.json order)."""
    from concourse import hw_specs
    tables = hw_specs.get_activation_tables(arch)  # cached dict, mutate in place
    mine = {AF.Ln, AF.Exp, AF.Square, AF.Copy, AF.Identity}
    for name, funcs in tables.items():
        if name != "natural_log_exp_and_others":
            funcs -= mine


def _build_nc():
    global OPS
    OPS = _register_custom_dve_ops()
    nc = bacc.Bacc("TRN2", debug=False)
    _pin_act_table_set(nc.m.arch)

    XS = nc.dram_tensor("XS", [C, 192], F32, kind="ExternalInput").ap()
    JS = nc.dram_tensor("JS", [C, 192], F32, kind="ExternalInput").ap()
    WMM = nc.dram_tensor("WMM", [6, 128, 128], F32, kind="ExternalInput").ap()
    OUT = nc.dram_tensor("OUT", [64, 3, C], F32, kind="ExternalOutput").ap()

    with tile.TileContext(nc) as tc, ExitStack() as ctx:
        const = ctx.enter_context(tc.tile_pool(name="const", bufs=1))
        io = ctx.enter_context(tc.tile_pool(name="io", bufs=2))
        sa = ctx.enter_context(tc.tile_pool(name="sa", bufs=1))
        prodp = ctx.enter_context(tc.tile_pool(name="prodp", bufs=2))
        rhsp = ctx.enter_context(tc.tile_pool(name="rhsp", bufs=2))
        xsbp = ctx.enter_context(tc.tile_pool(name="xsbp", bufs=2))
        s3p = ctx.enter_context(tc.tile_pool(name="s3p", bufs=1))
        outp = ctx.enter_context(tc.tile_pool(name="outp", bufs=2))
        psT = ctx.enter_context(tc.tile_pool(name="psT", bufs=1, space="PSUM"))
        psY = ctx.enter_context(tc.tile_pool(name="psY", bufs=1, space="PSUM"))
        psD = ctx.enter_context(tc.tile_pool(name="psD", bufs=2, space="PSUM"))

        # bias constants for ACT
        eps_c = const.tile([128, 1], F32, tag="eps_c")
        ln8_c = const.tile([128, 1], F32, tag="ln8_c")
        nc.gpsimd.memset(eps_c[:], EPS)
        nc.gpsimd.memset(ln8_c[:], float(math.log(1.0 - NEG)))
        sgn_c = const.tile([128, 1], U32, tag="sgn_c")
        nc.gpsimd.memset(sgn_c[:], 0x80000000)

        # weights + identity, loaded once
        wsb = const.tile([128, 6, 128], F32)
        nc.sync.dma_start(wsb[:], WMM.rearrange("n p m -> p n m"))
        LW_D = wsb[:, 4, :]      # blkdiag(WdT, WdT)
        IDT = wsb[:, 5, :]       # identity
        # f32r-rounded copies of the Y-matmul weights (PE full rate at N>=256)
        wsbr = const.tile([128, 4, 128], F32R, tag="wsbr")
        for k in range(4):
            nc.scalar.activation(wsbr[:, k, :], wsb[:, k, :], AF.Copy)
        LW_A = wsbr[:, 0, :]     # blkdiag(WaT, WaT)
        LW_2 = wsbr[:, 1, :]     # blkdiag((Wa-Wc).T, (Wc-Wa).T)
        LW_B = wsbr[:, 2, :]     # blkdiag(WbT, WbT)
        LW_1 = wsbr[:, 3, 0:64]  # [WaT; -WbT], M=64

        X3 = XS.rearrange("(u s p) w -> u p s w", p=128, s=NCHUNK)
        J3 = JS.rearrange("(u s p) w -> u p s w", p=128, s=NCHUNK)

        for u in range(NSUP):
            xs = io.tile([128, NCHUNK * 192], F32, tag="xs")
            js = io.tile([128, NCHUNK * 192], F32, tag="js")
            nc.sync.dma_start(
                xs[:].rearrange("p (s w) -> p s w", s=NCHUNK, w=192), X3[u])
            nc.sync.dma_start(
                js[:].rearrange("p (s w) -> p s w", s=NCHUNK, w=192), J3[u])
            xv = xs[:].rearrange("p (s e c) -> p s e c", s=NCHUNK, e=E, c=3)
            jv = js[:].rearrange("p (s e c) -> p s e c", s=NCHUNK, e=E, c=3)

            def v3(t):  # [128, 512] tile -> [128, 8, 64] view
                return t[:].rearrange("p (s e) -> p s e", s=NCHUNK, e=E)

            # ---- stage A: per-edge scalars gam, A1p --------------------
            q = sa.tile([128, SUPER // 2], F32, tag="q")
            n2 = sa.tile([128, SUPER // 2], F32, tag="n2")
            nc.vector._custom_dve(OPS["SQSUM"], out=v3(q),
                                  in0=jv[:, :, :, 0], in1=jv[:, :, :, 1])
            nc.vector._custom_dve(OPS["ADDSQ"], out=v3(n2),
                                  in0=v3(q), in1=jv[:, :, :, 2])
            # ln-domain: t = rsqrt(n2) = exp(-0.5 ln n2);
            # |A1| = sqrt(q/n2) = exp(0.5 (ln q - ln n2)); sign from gam+eps.
            # (dropping the +EPS inside t shifts gam by ~1e-6 rel: negligible)
            lq = sa.tile([128, SUPER // 2], F32, tag="lq")
            ln2 = sa.tile([128, SUPER // 2], F32, tag="ln2")
            nc.scalar.activation(lq[:], q[:], AF.Ln)
            nc.scalar.activation(ln2[:], n2[:], AF.Ln)
            t_ = sa.tile([128, SUPER // 2], F32, tag="t_")
            nc.scalar.activation(t_[:], ln2[:], AF.Exp, scale=-0.5)
            df = sa.tile([128, SUPER // 2], F32, tag="df")
            nc.vector.tensor_tensor(df[:], lq[:], ln2[:], ALU.subtract)
            rho = sa.tile([128, SUPER // 2], F32, tag="rho")
            nc.scalar.activation(rho[:], df[:], AF.Exp, scale=0.5)
            gam = sa.tile([128, SUPER // 2], F32, tag="gam")
            nc.vector.tensor_tensor(v3(gam), jv[:, :, :, 2], v3(t_), ALU.mult)
            h = sa.tile([128, SUPER // 2], F32, tag="h")
            nc.vector.tensor_scalar(h[:], gam[:], EPS, None, ALU.add)
            a1 = sa.tile([128, SUPER // 2], F32, tag="a1")
            nc.vector.scalar_tensor_tensor(
                a1[:].bitcast(U32), h[:].bitcast(U32), sgn_c[:],
                rho[:].bitcast(U32), ALU.bitwise_and, ALU.bitwise_or)

            # ---- stage B: feature blocks into PROD slots ---------------
            # slots: 0=Xx 1=Xz 2=A1p*w 3=gam*w 4=gam*Xy 5=A1p*Xy 6=Xy 7=c8
            prod = prodp.tile([128, NCHUNK, 8, E], F32, tag="prod")
            nc.gpsimd.tensor_copy(prod[:, :, 0, :], xv[:, :, :, 0])
            nc.gpsimd.tensor_copy(prod[:, :, 1, :], xv[:, :, :, 2])
            nc.gpsimd.tensor_copy(prod[:, :, 6, :], xv[:, :, :, 1])
            m1 = sa.tile([128, SUPER // 2], F32, tag="m1")
            m2 = sa.tile([128, SUPER // 2], F32, tag="m2")
            wt = sa.tile([128, SUPER // 2], F32, tag="wt")
            nc.gpsimd.tensor_tensor(v3(m1), v3(gam), xv[:, :, :, 2], ALU.mult)
            nc.gpsimd.tensor_tensor(v3(m2), v3(a1), xv[:, :, :, 0], ALU.mult)
            nc.vector.tensor_tensor(wt[:], m1[:], m2[:], ALU.subtract)
            nc.vector.tensor_tensor(prod[:, :, 2, :], v3(a1), v3(wt), ALU.mult)
            nc.vector.tensor_tensor(prod[:, :, 3, :], v3(gam), v3(wt), ALU.mult)
            nc.gpsimd.tensor_tensor(prod[:, :, 4, :], v3(gam), xv[:, :, :, 1],
                                    ALU.mult)
            nc.gpsimd.tensor_tensor(prod[:, :, 5, :], v3(a1), xv[:, :, :, 1],
                                    ALU.mult)
            m3 = sa.tile([128, SUPER // 2], F32, tag="m3")
            m4 = sa.tile([128, SUPER // 2], F32, tag="m4")
            nc.gpsimd.tensor_tensor(v3(m3), v3(a1), xv[:, :, :, 2], ALU.mult)
            nc.gpsimd.tensor_tensor(v3(m4), v3(gam), xv[:, :, :, 0], ALU.mult)
            nc.vector.tensor_tensor(prod[:, :, 7, :], v3(m3), v3(m4), ALU.add)

            # ---- per group: transpose, matmuls, Y copies ---------------
            xsb = xsbp.tile([128, 3, GROUP], F32, tag="xsb")
            for g in range(2):
                tpX = psT.tile([128, GROUP], F32, tag="tpX")
                tp1 = psT.tile([128, GROUP], F32, tag="tp1")
                tp2 = psT.tile([128, GROUP], F32, tag="tp2")
                tp3 = psT.tile([128, GROUP], F32, tag="tp3")
                for k in range(4):
                    s = 4 * g + k
                    sl = slice(128 * k, 128 * (k + 1))
                    nc.tensor.transpose(tpX[:, sl], prod[:, s, 0:2, :], IDT)
                    nc.tensor.transpose(tp1[:, sl], prod[:, s, 2:4, :], IDT)
                    nc.tensor.transpose(tp2[:, sl], prod[:, s, 4:6, :], IDT)
                    nc.tensor.transpose(tp3[:, sl], prod[:, s, 6:8, :], IDT)
                rhX = rhsp.tile([128, GROUP], F32R, tag="rhX")
                rh1 = rhsp.tile([128, GROUP], F32R, tag="rh1")
                rh2 = rhsp.tile([128, GROUP], F32R, tag="rh2")
                rh3 = rhsp.tile([128, GROUP], F32R, tag="rh3")
                nc.scalar.activation(rhX[:], tpX[:], AF.Copy)
                nc.scalar.activation(rh1[:], tp1[:], AF.Copy)
                nc.scalar.activation(rh2[:], tp2[:], AF.Copy)
                nc.scalar.activation(rh3[:], tp3[:], AF.Copy)

                pA = psY.tile([128, GROUP], F32, tag="pA")
                pB = psY.tile([64, GROUP], F32, tag="pB")
                nc.tensor.matmul(pA[:], LW_A, rhX[:], start=True, stop=False)
                nc.tensor.matmul(pA[:], LW_2, rh1[:], start=False, stop=False)
                nc.tensor.matmul(pA[:], LW_B, rh2[:], start=False, stop=True)
                nc.tensor.matmul(pB[:], LW_1, rh3[:], start=True, stop=True)

                ro = slice(64 * g, 64 * (g + 1))
                nc.scalar.activation(xsb[ro, 0, :], pA[0:64, :], AF.Copy)
                nc.vector.tensor_copy(xsb[ro, 2, :], pA[64:128, :])
                nc.scalar.activation(xsb[ro, 1, :], pB[:], AF.Copy)

            # ---- Wd stage + VN leaky relu ------------------------------
            dsb = s3p.tile([128, 3, GROUP], F32, tag="dsb")
            for i in range(3):
                pd = psD.tile([128, GROUP], F32, tag="pd")
                nc.tensor.matmul(pd[:], LW_D, xsb[:, i, :], start=True,
                                 stop=True)
                nc.scalar.activation(dsb[:, i, :], pd[:], AF.Copy)

            xd0 = s3p.tile([128, GROUP], F32, tag="xd0")
            xd1 = s3p.tile([128, GROUP], F32, tag="xd1")
            xd2 = s3p.tile([128, GROUP], F32, tag="xd2")
            dot = s3p.tile([128, GROUP], F32, tag="dot")
            nc.gpsimd.tensor_tensor(xd0[:], xsb[:, 0, :], dsb[:, 0, :],
                                    ALU.mult)
            nc.gpsimd.tensor_tensor(xd1[:], xsb[:, 1, :], dsb[:, 1, :],
                                    ALU.mult)
            nc.gpsimd.tensor_tensor(xd2[:], xsb[:, 2, :], dsb[:, 2, :],
                                    ALU.mult)
            nc.vector.tensor_tensor(dot[:], xd0[:], xd1[:], ALU.add)
            nc.vector.tensor_tensor(dot[:], dot[:], xd2[:], ALU.add)

            dn2 = s3p.tile([128, GROUP], F32, tag="dn2")
            nc.vector._custom_dve(OPS["SQSUM"], out=dn2[:],
                                  in0=dsb[:, 0, :], in1=dsb[:, 1, :])
            nc.vector._custom_dve(OPS["ADDSQ"], out=dn2[:],
                                  in0=dn2[:], in1=dsb[:, 2, :])

            lnv = s3p.tile([128, GROUP], F32, tag="lnv")
            rec = s3p.tile([128, GROUP], F32, tag="rec")
            nc.scalar.activation(lnv[:], dn2[:], AF.Ln, bias=eps_c[:])
            nc.scalar.activation(rec[:], lnv[:], AF.Exp, scale=-1.0,
                                 bias=ln8_c[:])
            s2 = s3p.tile([128, GROUP], F32, tag="s2")
            nc.vector.scalar_tensor_tensor(s2[:], dot[:], 0.0, rec[:],
                                           ALU.min, ALU.mult)

            ot = outp.tile([128, 3, GROUP], F32, tag="ot")
            for i in range(3):
                mi = s3p.tile([128, GROUP], F32, tag=f"mi{i}")
                nc.gpsimd.tensor_tensor(mi[:], s2[:], dsb[:, i, :], ALU.mult)
                nc.vector.tensor_tensor(ot[:, i, :], xsb[:, i, :], mi[:],
                                        ALU.subtract)

            c0 = u * SUPER
            nc.sync.dma_start(OUT[:, :, c0:c0 + GROUP], ot[0:64])
            nc.sync.dma_start(OUT[:, :, c0 + GROUP:c0 + SUPER], ot[64:128])

    nc.compile()
    return nc


_NC = None


def _get_nc():
    global _NC
    if _NC is None:
        _NC = _build_nc()
    return _NC


def _weight_stack(Wa, Wb, Wc, Wd):
    Z = np.zeros((64, 64), np.float32)

    def blk(a, b):
        return np.block([[a, Z], [Z, b]]).astype(np.float32)

    WaT = Wa.T.astype(np.float32)
    WbT = Wb.T.astype(np.float32)
    W2nT = (Wa - Wc).T.astype(np.float32)
    W2T = (Wc - Wa).T.astype(np.float32)
    WdT = Wd.T.astype(np.float32)
    w = np.stack([
        blk(WaT, WaT),
        blk(W2nT, W2T),
        blk(WbT, WbT),
        np.block([[WaT, Z], [-WbT, Z]]).astype(np.float32),
        blk(WdT, WdT),
        np.eye(128, dtype=np.float32),
    ])
    return np.ascontiguousarray(w, np.float32)


def run_full(X, J, Wa, Wb, Wc, Wd, trace=False, trace_kwargs=None):
    nc = _get_nc()
    wmm = _weight_stack(Wa, Wb, Wc, Wd)
    in_maps = []
    for b in range(B):
        in_maps.append({
            "XS": np.ascontiguousarray(X[b].reshape(C, 192), np.float32),
            "JS": np.ascontiguousarray(J[b].reshape(C, 192), np.float32),
            "WMM": wmm,
        })
    res = bass_utils.run_bass_kernel_spmd(
        nc, in_maps, core_ids=list(range(B)), trace=trace,
        **(trace_kwargs or {}))
    out = np.stack([res.results[b]["OUT"] for b in range(B)])
    return out.astype(np.float32), res


def kernel(X, J, Wa, Wb, Wc, Wd):
    out, _ = run_full(X, J, Wa, Wb, Wc, Wd)
    return out
